# revision 37
# baseline (speedup 1.0000x reference)
"""Multi-head causal attention on 8 Trainium2 NeuronCores.

Problem: B=4, S=2048, d_model=512, H=8 heads, d_k=64, fp32, causal,
scale = 1/sqrt(d_model) (faithful source quirk).

Sharding: 32 (batch, head-group) units -> core c handles batch c%4 and
head group c//4 (4 heads = 256 projection columns). Each core computes
q/k/v projections for its column slice, causal attention for its 4
heads, and a partial output projection (its 256 rows of Wo). The host
sums the two partials per batch and adds the output bias.

v2 restructure (from trace analysis of the 191us baseline):
- The ACT (scalar) engine is the hard floor: 96 exp blocks ~ 83us at
  its 153 G elem/s peak. Everything else must stay off its queue, so
  the only scalar-queue DMAs are WQ/QT, which complete before the
  first exp issues.
- Phase B is software-pipelined depth 2: scores(kb) is emitted before
  PV(kb-1), so the PE fills the exp latency and the ACT engine never
  waits on a scores block.
- q/k head-duplication (K=128 contraction keeps the PE's activity
  monitor at full clock) is done with 16 big [64,2048] SBUF->SBUF
  DMAs split across the sync and gpsimd queues.
- Normalize broadcasts the PSUM sums row first (gpsimd), then runs
  reciprocal on 64 lanes and a fused PSUM-read multiply (vector),
  replacing the 1-lane reciprocal + extra copies.
"""

import sys

sys.path.insert(0, "/opt/trn_rl_repo")

from contextlib import ExitStack

import numpy as np

import concourse.bass as bass
import concourse.tile as tile
from concourse import bacc, mybir
from concourse.bass_utils import run_bass_kernel_spmd

FP32 = mybir.dt.float32
FP16 = mybir.dt.float16
MM = FP16  # matmul operand dtype
MM_NP = np.float16
AF = mybir.ActivationFunctionType

B, S, DM, H = 4, 2048, 512, 8
DK = DM // H  # 64
HC = 4  # heads per core
COLS = HC * DK  # 256
P = 128
NKB = S // P  # 16 key blocks
SCALE = 1.0 / float(np.sqrt(np.float32(DM)))

_CACHED_NC = None


def _split512(w):
    """split [0, w) into chunks of <=512"""
    out = []
    lo = 0
    while lo < w:
        hi = min(lo + 512, w)
        out.append((lo, hi))
        lo = hi
    return out


def build_program():
    nc = bacc.Bacc("TRN2", target_bir_lowering=False, debug=False)

    qt_d = nc.dram_tensor("QT", [DM, S], MM, kind="ExternalInput").ap()
    kt_d = nc.dram_tensor("KT", [DM, S], MM, kind="ExternalInput").ap()
    vt_d = nc.dram_tensor("VT", [DM, S], MM, kind="ExternalInput").ap()
    wq_d = nc.dram_tensor("WQ", [P, DM // P, COLS], MM, kind="ExternalInput").ap()
    wk_d = nc.dram_tensor("WK", [P, DM // P, COLS], MM, kind="ExternalInput").ap()
    wv_d = nc.dram_tensor("WV", [P, DM // P, COLS], MM, kind="ExternalInput").ap()
    wo_d = nc.dram_tensor("WO", [P, COLS // P, DM], MM, kind="ExternalInput").ap()
    # small constants are packed into two >=512B-per-partition tensors:
    # sub-512B descriptors make the SDMA do read-modify-write on SBUF,
    # which can lose concurrent writes from other engines.
    idnmsk_d = nc.dram_tensor("IDNMSK", [P, 2, P], MM, kind="ExternalInput").ap()
    bqkv_d = nc.dram_tensor("BQKV", [P, 4 + COLS], FP32, kind="ExternalInput").ap()
    out_d = nc.dram_tensor("OUT", [S, DM], MM, kind="ExternalOutput").ap()

    with tile.TileContext(nc) as tc, ExitStack() as ctx:
        const = ctx.enter_context(tc.tile_pool(name="const", bufs=1))
        persist = ctx.enter_context(tc.tile_pool(name="persist", bufs=1))

        # ---- constants and input streams.
        # DMA-completion semaphores come from 8 global lanes assigned in
        # EMISSION order, so the exp-critical transfers (WQ/QT-head on the
        # scalar ring, WK/KT/QT-tail on sync) are emitted first and own
        # the early lanes; VT and the leftovers recycle lanes as they
        # free.  QT and VT slab-pairs are merged into single dma_starts
        # (fewer lanes, same fat descriptors).
        xin = ctx.enter_context(tc.tile_pool(name="xin", bufs=4))
        wq_sb = const.tile([P, DM // P, COLS], MM, tag="wq")
        nc.scalar.dma_start(out=wq_sb[:], in_=wq_d[:, :, :])
        qt_hd = xin.tile([P, 2, S], MM, tag="xq", bufs=2, name="qt_hd")
        nc.scalar.dma_start(
            out=qt_hd[:], in_=qt_d.rearrange("(j p) s -> p j s", p=P)[:, 2:4, :]
        )
        wk_sb = const.tile([P, DM // P, COLS], MM, tag="wk")
        nc.sync.dma_start(out=wk_sb[:], in_=wk_d[:, :, :])
        kt_tiles = []
        for j in range(DM // P):
            xt = xin.tile([P, S], MM, tag="xk", bufs=4, name=f"kt{j}")
            nc.sync.dma_start(out=xt[:], in_=kt_d[j * P : (j + 1) * P, :])
            kt_tiles.append(xt)
        # VT is emitted eighth so it gets a first-round DMA semaphore
        # lane (lanes recycle only when their consumer waits), landing
        # before the V projection reaches the head of the PE queue.
        vt_all = xin.tile([P, DM // P, S], MM, tag="xv", bufs=1, name="vt_all")
        nc.sync.dma_start(out=vt_all[:], in_=vt_d.rearrange("(j p) s -> p j s", p=P))
        # QT tail + WV ride the SWDGE (gpsimd) ring: separate semaphore
        # pool from the 8 HWDGE lanes, so they land without waiting for
        # a lane to recycle.
        qt_tl = xin.tile([P, 2, S], MM, tag="xq", bufs=2, name="qt_tl")
        nc.gpsimd.dma_start(
            out=qt_tl[:], in_=qt_d.rearrange("(j p) s -> p j s", p=P)[:, 0:2, :]
        )
        wv_sb = const.tile([P, DM // P, COLS], MM, tag="wv")
        nc.gpsimd.dma_start(out=wv_sb[:], in_=wv_d[:, :, :])
        # Q projection accumulates the early-landing slabs first:
        # (tile, row) pairs in contraction order j = 2, 3, 0, 1.
        qt_slab = [(qt_tl, 0), (qt_tl, 1), (qt_hd, 0), (qt_hd, 1)]
        qj_order = [2, 3, 0, 1]

        # small constants (packed; see dram tensor comment)
        idnmsk_sb = const.tile([P, 2, P], MM, tag="idnmsk")
        nc.sync.dma_start(out=idnmsk_sb[:], in_=idnmsk_d[:, :, :])
        idn_sb = idnmsk_sb[:, 0, :]
        msk_sb = idnmsk_sb[:, 1, :]
        bqkv_sb = const.tile([P, 4 + COLS], FP32, tag="bqkv")
        nc.gpsimd.dma_start(out=bqkv_sb[:], in_=bqkv_d[:, :])
        bq_sb = bqkv_sb[:, 0:2]
        bk_sb = bqkv_sb[:, 2:4]
        # 3D per-(head, dim) view of the v bias for the one-shot strided
        # V copyback add
        bvb3_sb = const.tile([P, HC, DK], FP32, tag="bvb3")
        nc.gpsimd.dma_start(out=bvb3_sb[:], in_=bqkv_d[:, 4 : 4 + COLS])

        # ---- persistent activations ----
        qt_st = [persist.tile([P, S], MM, tag=f"qst{i}", name=f"qst{i}") for i in range(2)]
        kt_st = [persist.tile([P, S], MM, tag=f"kst{i}", name=f"kst{i}") for i in range(2)]
        v_sb = persist.tile([P, NKB, HC, DK + 1], MM, tag="vaug")
        # ones column of v_aug (PV's 65th row yields the softmax sums):
        # written by the vector engine -- a DMA here would use 2-byte
        # descriptors whose read-modify-write races the v copybacks.
        nc.vector.memset(v_sb[:, :, :, DK : DK + 1], 1.0)
        wo_sb = const.tile([P, COLS // P, DM], MM, tag="wo")
        ctxt_sb = [persist.tile([P, S], MM, tag=f"ctxt{i}", name=f"ctxt{i}") for i in range(2)]
        # Per-head q/k with the 64 head dims DUPLICATED onto both partition
        # halves: scores contract over K=128, which the PE's activity
        # monitor needs to hold the 2.4 GHz clock. The doubled dot product
        # is folded into the exp scale.
        qt_dup = [persist.tile([P, S], MM, tag=f"qtd{h}", name=f"qtd{h}") for h in range(HC)]
        kt_dup = [persist.tile([P, S], MM, tag=f"ktd{h}", name=f"ktd{h}") for h in range(HC)]

        def dup_head(st, dup, h, eng):
            """one [64, 2048] sbuf->sbuf copy per partition half"""
            s_ap = st[h // 2][(h % 2) * DK : (h % 2) * DK + DK, :]
            for half in range(2):
                eng.dma_start(out=dup[h][half * DK : (half + 1) * DK, :], in_=s_ap)

        # ================= Phase A: projections =================
        # PE warmup: the HAM clock gate defaults to 1.2 GHz and needs
        # ~3.4us of sustained matmul activity to release 2.4 GHz.  The PE
        # is DMA-starved for the first ~12us anyway, so burn it on junk
        # matmuls over a memset scratch: the projections then start warm.
        warm_sb = persist.tile([P, 512], MM, tag="warm")
        nc.vector.memset(warm_sb[:], 0.25)
        with tc.tile_pool(name="pj_psum", bufs=8, space="PSUM") as pj_psum:
            wps = pj_psum.tile([P, 512], FP32, tag="ps", name="warm_ps")
            for i in range(10):
                nc.tensor.matmul(
                    wps[:],
                    warm_sb[:, :P],
                    warm_sb[:],
                    start=True,
                    stop=True,
                    skip_group_check=True,
                )
            # K: j-outer over all 8 (cc,t) accumulators so the first matmul
            # starts as soon as KT slab 0 lands.
            pss = [
                pj_psum.tile([P, 512], FP32, tag="ps", name=f"psk{i}")
                for i in range(8)
            ]
            for j in range(DM // P):
                for cc in range(COLS // P):
                    for t in range(S // 512):
                        nc.tensor.matmul(
                            pss[cc * 4 + t][:],
                            wk_sb[:, j, cc * P : (cc + 1) * P],
                            kt_tiles[j][:, t * 512 : (t + 1) * 512],
                            start=(j == 0),
                            stop=(j == DM // P - 1),
                        )
            for cc in range(COLS // P):
                for t in range(S // 512):
                    ts_ = slice(t * 512, (t + 1) * 512)
                    # split across the (idle) ACT engine and the DVE so
                    # the copyback wall is half as long
                    if t < 2:
                        nc.scalar.add(
                            kt_st[cc][:, ts_], pss[cc * 4 + t][:], bk_sb[:, cc : cc + 1]
                        )
                    else:
                        nc.vector.tensor_scalar_add(
                            kt_st[cc][:, ts_], pss[cc * 4 + t][:], bk_sb[:, cc : cc + 1]
                        )
                for hh in range(2):
                    dup_head(kt_st, kt_dup, cc * 2 + hh, nc.gpsimd)

            # Q: j-outer like K, accumulating the early-landing QT slabs
            # (2,3 on the scalar ring) before the sync-ring tail (0,1).
            pssq = [
                pj_psum.tile([P, 512], FP32, tag="ps", name=f"psq{i}")
                for i in range(8)
            ]
            for ji, j in enumerate(qj_order):
                q_tile, q_row = qt_slab[j]
                for cc in range(COLS // P):
                    for t in range(S // 512):
                        nc.tensor.matmul(
                            pssq[cc * 4 + t][:],
                            wq_sb[:, j, cc * P : (cc + 1) * P],
                            q_tile[:, q_row, t * 512 : (t + 1) * 512],
                            start=(ji == 0),
                            stop=(ji == DM // P - 1),
                        )
            for cc in range(COLS // P):
                for t in range(S // 512):
                    ts_ = slice(t * 512, (t + 1) * 512)
                    if t < 2:
                        nc.scalar.add(
                            qt_st[cc][:, ts_], pssq[cc * 4 + t][:], bq_sb[:, cc : cc + 1]
                        )
                    else:
                        nc.vector.tensor_scalar_add(
                            qt_st[cc][:, ts_], pssq[cc * 4 + t][:], bq_sb[:, cc : cc + 1]
                        )
                for hh in range(2):
                    dup_head(qt_st, qt_dup, cc * 2 + hh, nc.gpsimd)
            nc.gpsimd.dma_start(out=wo_sb[:], in_=wo_d[:, :, :])

            # V projection (t-outer): natural layout, one strided bias-add
            # copyback per tb covering all 4 heads.
            for tb in range(NKB):
                ps = pj_psum.tile([P, HC, DK], FP32, tag="ps", name=f"vps{tb}")
                for j in range(DM // P):
                    nc.tensor.matmul(
                        ps[:, :, :],
                        vt_all[:, j, tb * P : (tb + 1) * P],
                        wv_sb[:, j, :],
                        start=(j == 0),
                        stop=(j == DM // P - 1),
                    )
                nc.vector.tensor_add(
                    v_sb[:, tb, :, 0:DK], ps[:, :, :], bvb3_sb[:, :, :]
                )

        # ================= Phase B: attention, software-pipelined =========
        # Flat (h, kb) stream with PV lagging scores by one kb: while the
        # ACT engine runs exp(kb), the PE runs PV(kb-1) and scores(kb+1),
        # so the scalar engine (the critical resource at ~83us) never
        # stalls on the PE.  PV accumulates into four per-head QUARTER
        # tiles [65, 512] (one PSUM bank each): quarter qi closes at
        # kb = 4*qi+3, so normalization and the last head's output
        # projections start mid-head instead of after its final PV.
        with tc.tile_pool(name="pt", bufs=6) as pt_pool, tc.tile_pool(
            name="sc_psum", bufs=2, space="PSUM"
        ) as sc_psum, tc.tile_pool(
            name="out_psum", bufs=4, space="PSUM"
        ) as out_psum, tc.tile_pool(
            name="norm", bufs=4
        ) as norm_pool, tc.tile_pool(name="osb", bufs=4) as osb:

            def oproj(tb, pin, cast_eng=None):
                ps = out_psum.tile([P, DM], FP32, tag="po", name=f"ops{tb}")
                if pin:
                    # advance the tag ring past the live quarter slots so
                    # the next oproj reuses this (already-freed) slot
                    for i in range(3):
                        out_psum.tile([1, 1], FP32, tag="po", name=f"pin{tb}_{i}")
                for cc in range(COLS // P):
                    nc.tensor.matmul(
                        ps[:],
                        ctxt_sb[cc][:, tb * P : (tb + 1) * P],
                        wo_sb[:, cc, :],
                        start=(cc == 0),
                        stop=(cc == COLS // P - 1),
                    )
                o = osb.tile([P, DM], MM, tag="o", name=f"o{tb}")
                if cast_eng == "scalar":
                    nc.scalar.copy(o[:], ps[:])
                else:
                    nc.vector.tensor_copy(o[:], ps[:])
                eng = nc.sync if tb % 2 == 0 else nc.gpsimd
                eng.dma_start(out=out_d[tb * P : (tb + 1) * P, :], in_=o[:])

            def normalize(po, qi, ti, po_):
                """ctxT[head, quarter] = po[0:64] * (1 / po[64]).

                gpsimd broadcasts the sums row so the reciprocal runs on
                64 DVE lanes; the multiply reads the PSUM context rows
                directly and writes the fp16 ctxt slice."""
                nm = f"{ti}_{po_}_{qi}"
                sums = norm_pool.tile([1, 512], FP32, tag="sums", name=f"s{nm}")
                nc.vector.tensor_copy(sums[:], po[DK : DK + 1, :])
                bcast = norm_pool.tile([DK, 512], FP32, tag="bcast", name=f"b{nm}")
                nc.gpsimd.partition_broadcast(bcast[:], sums[:])
                recip = norm_pool.tile([DK, 512], FP32, tag="recip", name=f"r{nm}")
                nc.vector.reciprocal_approx_fast(out=recip[:], in_=bcast[:])
                nc.vector.tensor_mul(
                    ctxt_sb[ti][po_ : po_ + DK, qi * 512 : (qi + 1) * 512],
                    po[0:DK, :],
                    recip[:],
                )

            po_quarters = {}  # h -> [po_q0 .. po_q3]
            pts = {}  # (h, kb) -> list of (jt, tq0, w, pt_tile)

            def emit_scores_exp(h, kb):
                q0 = kb * P
                qt_h, kt_h = qt_dup[h], kt_dup[h]
                blocks = []
                for jt in range(q0 // 1024, S // 1024):
                    tq0 = max(q0, 1024 * jt)
                    w = 1024 * (jt + 1) - tq0
                    sc = sc_psum.tile([P, 1024], FP32, tag="sc", name=f"sc{h}_{kb}_{jt}")
                    if tq0 == q0:
                        # diagonal block: seed the first 128 columns with
                        # the additive causal mask BEFORE the score
                        # matmuls, so the exp is never tail-latched by
                        # the mask's weight load.
                        nc.tensor.matmul(
                            sc[:, 0:P],
                            idn_sb,
                            msk_sb,
                            start=True,
                            stop=False,
                            skip_group_check=True,
                        )
                    for lo, hi in _split512(w):
                        nc.tensor.matmul(
                            sc[:, lo:hi],
                            kt_h[:, q0 : q0 + P],
                            qt_h[:, tq0 + lo : tq0 + hi],
                            start=(tq0 != q0 or lo > 0),
                            stop=True,
                            skip_group_check=True,
                        )
                    pt = pt_pool.tile([P, 1024], MM, tag="pt", name=f"pt{h}_{kb}_{jt}")
                    nc.scalar.activation(
                        pt[:, :w],
                        sc[:, :w],
                        AF.Exp,
                        scale=SCALE / 2.0,
                    )
                    blocks.append((jt, tq0, w, pt))
                pts[(h, kb)] = blocks

            def emit_pv(h, kb):
                po_q = po_quarters[h]
                for jt, tq0, w, pt in pts.pop((h, kb)):
                    qc = (tq0 // 512) * 512
                    while qc < tq0 + w:
                        glo, ghi = max(tq0, qc), min(tq0 + w, qc + 512)
                        qi = qc // 512
                        nc.tensor.matmul(
                            po_q[qi][:, glo - qc : ghi - qc],
                            v_sb[:, kb, h, :],
                            pt[:, glo - tq0 : ghi - tq0],
                            start=(kb == 0),
                            stop=(kb == 4 * qi + 3),
                            skip_group_check=True,
                        )
                        qc += 512

            pending = None  # (h, kb) whose PV is not yet emitted
            for h in range(HC):
                ti, po_ = h // 2, (h % 2) * DK
                po_quarters[h] = [
                    out_psum.tile([DK + 1, 512], FP32, tag="po", name=f"po{h}_{qi}")
                    for qi in range(4)
                ]
                for kb in range(NKB):
                    emit_scores_exp(h, kb)
                    if pending is not None:
                        ph, pkb = pending
                        emit_pv(ph, pkb)
                        pti, ppo_ = ph // 2, (ph % 2) * DK
                        if pkb % 4 == 3:  # quarter pkb//4 just closed
                            normalize(po_quarters[ph][pkb // 4], pkb // 4, pti, ppo_)
                        # last head: trickle one output projection per
                        # iteration as soon as its quarter normalizes
                        # (tb k-4 needs quarter (k-4)//4, closed at
                        # iteration 4*((k-4)//4)+4 <= k).
                        if ph == HC - 1 and pkb >= 4:
                            oproj(pkb - 4, pin=True)
                    pending = (h, kb)
            # flush: last PV closes quarter 3 of the last head; normalize
            # it and emit the remaining output projections.
            emit_pv(HC - 1, NKB - 1)
            oproj(11, pin=True)
            normalize(
                po_quarters[HC - 1][3], 3, (HC - 1) // 2, ((HC - 1) % 2) * DK
            )
            for tb in range(12, NKB):
                oproj(tb, pin=False)

    nc.compile()
    return nc


def _get_nc():
    global _CACHED_NC
    if _CACHED_NC is None:
        _CACHED_NC = build_program()
    return _CACHED_NC


def make_in_maps(Q, K, V, Wq, bq, Wk, bk, Wv, bv, Wo, bo):
    f32 = lambda a: np.ascontiguousarray(a, dtype=np.float32)
    mm = lambda a: np.ascontiguousarray(np.asarray(a), dtype=MM_NP)

    def pack_w(W, cs):
        """[512, 256] slice -> partition-contiguous [128, 4, 256]"""
        w = np.asarray(W)[:, cs].reshape(DM // P, P, COLS).transpose(1, 0, 2)
        return mm(w)

    def pack_o(W, cs):
        """[256, 512] slice -> partition-contiguous [128, 2, 512]"""
        w = np.asarray(W)[cs, :].reshape(COLS // P, P, DM).transpose(1, 0, 2)
        return mm(w)

    def pack_b(b, cs):
        return f32(np.asarray(b)[cs].reshape(COLS // P, P).T)
    qt = [mm(np.asarray(Q[b]).T) for b in range(B)]
    kt = [mm(np.asarray(K[b]).T) for b in range(B)]
    vt = [mm(np.asarray(V[b]).T) for b in range(B)]
    idnmsk = np.empty((P, 2, P), dtype=MM_NP)
    idnmsk[:, 0, :] = np.eye(P, dtype=MM_NP)
    idnmsk[:, 1, :] = np.tril(
        np.full((P, P), -30000.0, dtype=np.float32), -1
    ).astype(MM_NP)
    maps = []
    for c in range(8):
        b, hg = c % B, c // B
        cs = slice(hg * COLS, (hg + 1) * COLS)
        maps.append(
            {
                "QT": qt[b],
                "KT": kt[b],
                "VT": vt[b],
                "WQ": pack_w(Wq, cs),
                "WK": pack_w(Wk, cs),
                "WV": pack_w(Wv, cs),
                "WO": pack_o(Wo, cs),
                "IDNMSK": idnmsk,
                "BQKV": np.concatenate(
                    [
                        pack_b(bq, cs),
                        pack_b(bk, cs),
                        np.broadcast_to(f32(bv[cs]), (P, COLS)),
                    ],
                    axis=1,
                ).astype(np.float32),
            }
        )
    return maps


def assemble(results, bo):
    out = np.empty((B, S, DM), dtype=np.float32)
    for b in range(B):
        out[b] = results[b]["OUT"].astype(np.float32) + results[b + B][
            "OUT"
        ].astype(np.float32)
    out += np.asarray(bo, dtype=np.float32)
    return out


def kernel(Q, K, V, Wq, bq, Wk, bk, Wv, bv, Wo, bo):
    nc = _get_nc()
    maps = make_in_maps(Q, K, V, Wq, bq, Wk, bk, Wv, bv, Wo, bo)
    res = run_bass_kernel_spmd(nc, maps, list(range(8)))
    return assemble(res.results, bo)


# revision 38
# speedup vs baseline: 1.0006x; 1.0006x over previous
"""Multi-head causal attention on 8 Trainium2 NeuronCores.

Problem: B=4, S=2048, d_model=512, H=8 heads, d_k=64, fp32, causal,
scale = 1/sqrt(d_model) (faithful source quirk).

Sharding: 32 (batch, head-group) units -> core c handles batch c%4 and
head group c//4 (4 heads = 256 projection columns). Each core computes
q/k/v projections for its column slice, causal attention for its 4
heads, and a partial output projection (its 256 rows of Wo). The host
sums the two partials per batch and adds the output bias.

v2 restructure (from trace analysis of the 191us baseline):
- The ACT (scalar) engine is the hard floor: 96 exp blocks ~ 83us at
  its 153 G elem/s peak. Everything else must stay off its queue, so
  the only scalar-queue DMAs are WQ/QT, which complete before the
  first exp issues.
- Phase B is software-pipelined depth 2: scores(kb) is emitted before
  PV(kb-1), so the PE fills the exp latency and the ACT engine never
  waits on a scores block.
- q/k head-duplication (K=128 contraction keeps the PE's activity
  monitor at full clock) is done with 16 big [64,2048] SBUF->SBUF
  DMAs split across the sync and gpsimd queues.
- Normalize broadcasts the PSUM sums row first (gpsimd), then runs
  reciprocal on 64 lanes and a fused PSUM-read multiply (vector),
  replacing the 1-lane reciprocal + extra copies.
"""

import sys

sys.path.insert(0, "/opt/trn_rl_repo")

from contextlib import ExitStack

import numpy as np

import concourse.bass as bass
import concourse.tile as tile
from concourse import bacc, mybir
from concourse.bass_utils import run_bass_kernel_spmd

FP32 = mybir.dt.float32
FP16 = mybir.dt.float16
MM = FP16  # matmul operand dtype
MM_NP = np.float16
AF = mybir.ActivationFunctionType

B, S, DM, H = 4, 2048, 512, 8
DK = DM // H  # 64
HC = 4  # heads per core
COLS = HC * DK  # 256
P = 128
NKB = S // P  # 16 key blocks
SCALE = 1.0 / float(np.sqrt(np.float32(DM)))

_CACHED_NC = None


def _split512(w):
    """split [0, w) into chunks of <=512"""
    out = []
    lo = 0
    while lo < w:
        hi = min(lo + 512, w)
        out.append((lo, hi))
        lo = hi
    return out


def build_program():
    nc = bacc.Bacc("TRN2", target_bir_lowering=False, debug=False)

    qt_d = nc.dram_tensor("QT", [DM, S], MM, kind="ExternalInput").ap()
    kt_d = nc.dram_tensor("KT", [DM, S], MM, kind="ExternalInput").ap()
    vt_d = nc.dram_tensor("VT", [DM, S], MM, kind="ExternalInput").ap()
    wq_d = nc.dram_tensor("WQ", [P, DM // P, COLS], MM, kind="ExternalInput").ap()
    wk_d = nc.dram_tensor("WK", [P, DM // P, COLS], MM, kind="ExternalInput").ap()
    wv_d = nc.dram_tensor("WV", [P, DM // P, COLS], MM, kind="ExternalInput").ap()
    wo_d = nc.dram_tensor("WO", [P, COLS // P, DM], MM, kind="ExternalInput").ap()
    # small constants are packed into two >=512B-per-partition tensors:
    # sub-512B descriptors make the SDMA do read-modify-write on SBUF,
    # which can lose concurrent writes from other engines.
    idnmsk_d = nc.dram_tensor("IDNMSK", [P, 2, P], MM, kind="ExternalInput").ap()
    bqkv_d = nc.dram_tensor("BQKV", [P, 4 + COLS], FP32, kind="ExternalInput").ap()
    out_d = nc.dram_tensor("OUT", [S, DM], MM, kind="ExternalOutput").ap()

    with tile.TileContext(nc) as tc, ExitStack() as ctx:
        const = ctx.enter_context(tc.tile_pool(name="const", bufs=1))
        persist = ctx.enter_context(tc.tile_pool(name="persist", bufs=1))

        # ---- constants and input streams.
        # DMA-completion semaphores come from 8 global lanes assigned in
        # EMISSION order, so the exp-critical transfers (WQ/QT-head on the
        # scalar ring, WK/KT/QT-tail on sync) are emitted first and own
        # the early lanes; VT and the leftovers recycle lanes as they
        # free.  QT and VT slab-pairs are merged into single dma_starts
        # (fewer lanes, same fat descriptors).
        xin = ctx.enter_context(tc.tile_pool(name="xin", bufs=4))
        wq_sb = const.tile([P, DM // P, COLS], MM, tag="wq")
        nc.scalar.dma_start(out=wq_sb[:], in_=wq_d[:, :, :])
        qt_hd = xin.tile([P, 2, S], MM, tag="xq", bufs=2, name="qt_hd")
        nc.scalar.dma_start(
            out=qt_hd[:], in_=qt_d.rearrange("(j p) s -> p j s", p=P)[:, 2:4, :]
        )
        wk_sb = const.tile([P, DM // P, COLS], MM, tag="wk")
        nc.sync.dma_start(out=wk_sb[:], in_=wk_d[:, :, :])
        kt_tiles = []
        for j in range(DM // P):
            xt = xin.tile([P, S], MM, tag="xk", bufs=4, name=f"kt{j}")
            nc.sync.dma_start(out=xt[:], in_=kt_d[j * P : (j + 1) * P, :])
            kt_tiles.append(xt)
        # VT is emitted eighth so it gets a first-round DMA semaphore
        # lane (lanes recycle only when their consumer waits), landing
        # before the V projection reaches the head of the PE queue.
        vt_all = xin.tile([P, DM // P, S], MM, tag="xv", bufs=1, name="vt_all")
        nc.sync.dma_start(out=vt_all[:], in_=vt_d.rearrange("(j p) s -> p j s", p=P))
        qt_tl = xin.tile([P, 2, S], MM, tag="xq", bufs=2, name="qt_tl")
        nc.sync.dma_start(
            out=qt_tl[:], in_=qt_d.rearrange("(j p) s -> p j s", p=P)[:, 0:2, :]
        )
        wv_sb = const.tile([P, DM // P, COLS], MM, tag="wv")
        nc.sync.dma_start(out=wv_sb[:], in_=wv_d[:, :, :])
        # Q projection accumulates the early-landing slabs first:
        # (tile, row) pairs in contraction order j = 2, 3, 0, 1.
        qt_slab = [(qt_tl, 0), (qt_tl, 1), (qt_hd, 0), (qt_hd, 1)]
        qj_order = [2, 3, 0, 1]

        # small constants (packed; see dram tensor comment)
        idnmsk_sb = const.tile([P, 2, P], MM, tag="idnmsk")
        nc.sync.dma_start(out=idnmsk_sb[:], in_=idnmsk_d[:, :, :])
        idn_sb = idnmsk_sb[:, 0, :]
        msk_sb = idnmsk_sb[:, 1, :]
        bqkv_sb = const.tile([P, 4 + COLS], FP32, tag="bqkv")
        nc.gpsimd.dma_start(out=bqkv_sb[:], in_=bqkv_d[:, :])
        bq_sb = bqkv_sb[:, 0:2]
        bk_sb = bqkv_sb[:, 2:4]
        # 3D per-(head, dim) view of the v bias for the one-shot strided
        # V copyback add
        bvb3_sb = const.tile([P, HC, DK], FP32, tag="bvb3")
        nc.gpsimd.dma_start(out=bvb3_sb[:], in_=bqkv_d[:, 4 : 4 + COLS])

        # ---- persistent activations ----
        qt_st = [persist.tile([P, S], MM, tag=f"qst{i}", name=f"qst{i}") for i in range(2)]
        kt_st = [persist.tile([P, S], MM, tag=f"kst{i}", name=f"kst{i}") for i in range(2)]
        v_sb = persist.tile([P, NKB, HC, DK + 1], MM, tag="vaug")
        # ones column of v_aug (PV's 65th row yields the softmax sums):
        # written by the vector engine -- a DMA here would use 2-byte
        # descriptors whose read-modify-write races the v copybacks.
        nc.vector.memset(v_sb[:, :, :, DK : DK + 1], 1.0)
        wo_sb = const.tile([P, COLS // P, DM], MM, tag="wo")
        ctxt_sb = [persist.tile([P, S], MM, tag=f"ctxt{i}", name=f"ctxt{i}") for i in range(2)]
        # Per-head q/k with the 64 head dims DUPLICATED onto both partition
        # halves: scores contract over K=128, which the PE's activity
        # monitor needs to hold the 2.4 GHz clock. The doubled dot product
        # is folded into the exp scale.
        qt_dup = [persist.tile([P, S], MM, tag=f"qtd{h}", name=f"qtd{h}") for h in range(HC)]
        kt_dup = [persist.tile([P, S], MM, tag=f"ktd{h}", name=f"ktd{h}") for h in range(HC)]

        def dup_head(st, dup, h, eng):
            """one [64, 2048] sbuf->sbuf copy per partition half"""
            s_ap = st[h // 2][(h % 2) * DK : (h % 2) * DK + DK, :]
            for half in range(2):
                eng.dma_start(out=dup[h][half * DK : (half + 1) * DK, :], in_=s_ap)

        # ================= Phase A: projections =================
        # PE warmup: the HAM clock gate defaults to 1.2 GHz and needs
        # ~3.4us of sustained matmul activity to release 2.4 GHz.  The PE
        # is DMA-starved for the first ~12us anyway, so burn it on junk
        # matmuls over a memset scratch: the projections then start warm.
        warm_sb = persist.tile([P, 512], MM, tag="warm")
        nc.vector.memset(warm_sb[:], 0.25)
        with tc.tile_pool(name="pj_psum", bufs=8, space="PSUM") as pj_psum:
            wps = pj_psum.tile([P, 512], FP32, tag="ps", name="warm_ps")
            for i in range(10):
                nc.tensor.matmul(
                    wps[:],
                    warm_sb[:, :P],
                    warm_sb[:],
                    start=True,
                    stop=True,
                    skip_group_check=True,
                )
            # K: j-outer over all 8 (cc,t) accumulators so the first matmul
            # starts as soon as KT slab 0 lands.
            pss = [
                pj_psum.tile([P, 512], FP32, tag="ps", name=f"psk{i}")
                for i in range(8)
            ]
            for j in range(DM // P):
                for cc in range(COLS // P):
                    for t in range(S // 512):
                        nc.tensor.matmul(
                            pss[cc * 4 + t][:],
                            wk_sb[:, j, cc * P : (cc + 1) * P],
                            kt_tiles[j][:, t * 512 : (t + 1) * 512],
                            start=(j == 0),
                            stop=(j == DM // P - 1),
                        )
            for cc in range(COLS // P):
                for t in range(S // 512):
                    ts_ = slice(t * 512, (t + 1) * 512)
                    # split across the (idle) ACT engine and the DVE so
                    # the copyback wall is half as long
                    if t < 2:
                        nc.scalar.add(
                            kt_st[cc][:, ts_], pss[cc * 4 + t][:], bk_sb[:, cc : cc + 1]
                        )
                    else:
                        nc.vector.tensor_scalar_add(
                            kt_st[cc][:, ts_], pss[cc * 4 + t][:], bk_sb[:, cc : cc + 1]
                        )
                for hh in range(2):
                    dup_head(kt_st, kt_dup, cc * 2 + hh, nc.gpsimd)

            # Q: j-outer like K, accumulating the early-landing QT slabs
            # (2,3 on the scalar ring) before the sync-ring tail (0,1).
            pssq = [
                pj_psum.tile([P, 512], FP32, tag="ps", name=f"psq{i}")
                for i in range(8)
            ]
            for ji, j in enumerate(qj_order):
                q_tile, q_row = qt_slab[j]
                for cc in range(COLS // P):
                    for t in range(S // 512):
                        nc.tensor.matmul(
                            pssq[cc * 4 + t][:],
                            wq_sb[:, j, cc * P : (cc + 1) * P],
                            q_tile[:, q_row, t * 512 : (t + 1) * 512],
                            start=(ji == 0),
                            stop=(ji == DM // P - 1),
                        )
            for cc in range(COLS // P):
                for t in range(S // 512):
                    ts_ = slice(t * 512, (t + 1) * 512)
                    if t < 2:
                        nc.scalar.add(
                            qt_st[cc][:, ts_], pssq[cc * 4 + t][:], bq_sb[:, cc : cc + 1]
                        )
                    else:
                        nc.vector.tensor_scalar_add(
                            qt_st[cc][:, ts_], pssq[cc * 4 + t][:], bq_sb[:, cc : cc + 1]
                        )
                for hh in range(2):
                    dup_head(qt_st, qt_dup, cc * 2 + hh, nc.gpsimd)
            nc.gpsimd.dma_start(out=wo_sb[:], in_=wo_d[:, :, :])

            # V projection (t-outer): natural layout, one strided bias-add
            # copyback per tb covering all 4 heads.
            for tb in range(NKB):
                ps = pj_psum.tile([P, HC, DK], FP32, tag="ps", name=f"vps{tb}")
                for j in range(DM // P):
                    nc.tensor.matmul(
                        ps[:, :, :],
                        vt_all[:, j, tb * P : (tb + 1) * P],
                        wv_sb[:, j, :],
                        start=(j == 0),
                        stop=(j == DM // P - 1),
                    )
                nc.vector.tensor_add(
                    v_sb[:, tb, :, 0:DK], ps[:, :, :], bvb3_sb[:, :, :]
                )

        # ================= Phase B: attention, software-pipelined =========
        # Flat (h, kb) stream with PV lagging scores by one kb: while the
        # ACT engine runs exp(kb), the PE runs PV(kb-1) and scores(kb+1),
        # so the scalar engine (the critical resource at ~83us) never
        # stalls on the PE.  PV accumulates into four per-head QUARTER
        # tiles [65, 512] (one PSUM bank each): quarter qi closes at
        # kb = 4*qi+3, so normalization and the last head's output
        # projections start mid-head instead of after its final PV.
        with tc.tile_pool(name="pt", bufs=6) as pt_pool, tc.tile_pool(
            name="sc_psum", bufs=2, space="PSUM"
        ) as sc_psum, tc.tile_pool(
            name="out_psum", bufs=4, space="PSUM"
        ) as out_psum, tc.tile_pool(
            name="norm", bufs=4
        ) as norm_pool, tc.tile_pool(name="osb", bufs=4) as osb:

            def oproj(tb, pin, cast_eng=None):
                ps = out_psum.tile([P, DM], FP32, tag="po", name=f"ops{tb}")
                if pin:
                    # advance the tag ring past the live quarter slots so
                    # the next oproj reuses this (already-freed) slot
                    for i in range(3):
                        out_psum.tile([1, 1], FP32, tag="po", name=f"pin{tb}_{i}")
                for cc in range(COLS // P):
                    nc.tensor.matmul(
                        ps[:],
                        ctxt_sb[cc][:, tb * P : (tb + 1) * P],
                        wo_sb[:, cc, :],
                        start=(cc == 0),
                        stop=(cc == COLS // P - 1),
                    )
                o = osb.tile([P, DM], MM, tag="o", name=f"o{tb}")
                if cast_eng == "scalar":
                    nc.scalar.copy(o[:], ps[:])
                else:
                    nc.vector.tensor_copy(o[:], ps[:])
                eng = nc.sync if tb % 2 == 0 else nc.gpsimd
                eng.dma_start(out=out_d[tb * P : (tb + 1) * P, :], in_=o[:])

            def normalize(po, qi, ti, po_):
                """ctxT[head, quarter] = po[0:64] * (1 / po[64]).

                gpsimd broadcasts the sums row so the reciprocal runs on
                64 DVE lanes; the multiply reads the PSUM context rows
                directly and writes the fp16 ctxt slice."""
                nm = f"{ti}_{po_}_{qi}"
                sums = norm_pool.tile([1, 512], FP32, tag="sums", name=f"s{nm}")
                nc.vector.tensor_copy(sums[:], po[DK : DK + 1, :])
                bcast = norm_pool.tile([DK, 512], FP32, tag="bcast", name=f"b{nm}")
                nc.gpsimd.partition_broadcast(bcast[:], sums[:])
                recip = norm_pool.tile([DK, 512], FP32, tag="recip", name=f"r{nm}")
                nc.vector.reciprocal_approx_fast(out=recip[:], in_=bcast[:])
                nc.vector.tensor_mul(
                    ctxt_sb[ti][po_ : po_ + DK, qi * 512 : (qi + 1) * 512],
                    po[0:DK, :],
                    recip[:],
                )

            po_quarters = {}  # h -> [po_q0 .. po_q3]
            pts = {}  # (h, kb) -> list of (jt, tq0, w, pt_tile)

            def emit_scores_exp(h, kb):
                q0 = kb * P
                qt_h, kt_h = qt_dup[h], kt_dup[h]
                blocks = []
                for jt in range(q0 // 1024, S // 1024):
                    tq0 = max(q0, 1024 * jt)
                    w = 1024 * (jt + 1) - tq0
                    sc = sc_psum.tile([P, 1024], FP32, tag="sc", name=f"sc{h}_{kb}_{jt}")
                    if tq0 == q0:
                        # diagonal block: seed the first 128 columns with
                        # the additive causal mask BEFORE the score
                        # matmuls, so the exp is never tail-latched by
                        # the mask's weight load.
                        nc.tensor.matmul(
                            sc[:, 0:P],
                            idn_sb,
                            msk_sb,
                            start=True,
                            stop=False,
                            skip_group_check=True,
                        )
                    for lo, hi in _split512(w):
                        nc.tensor.matmul(
                            sc[:, lo:hi],
                            kt_h[:, q0 : q0 + P],
                            qt_h[:, tq0 + lo : tq0 + hi],
                            start=(tq0 != q0 or lo > 0),
                            stop=True,
                            skip_group_check=True,
                        )
                    pt = pt_pool.tile([P, 1024], MM, tag="pt", name=f"pt{h}_{kb}_{jt}")
                    nc.scalar.activation(
                        pt[:, :w],
                        sc[:, :w],
                        AF.Exp,
                        scale=SCALE / 2.0,
                    )
                    blocks.append((jt, tq0, w, pt))
                pts[(h, kb)] = blocks

            def emit_pv(h, kb):
                po_q = po_quarters[h]
                for jt, tq0, w, pt in pts.pop((h, kb)):
                    qc = (tq0 // 512) * 512
                    while qc < tq0 + w:
                        glo, ghi = max(tq0, qc), min(tq0 + w, qc + 512)
                        qi = qc // 512
                        nc.tensor.matmul(
                            po_q[qi][:, glo - qc : ghi - qc],
                            v_sb[:, kb, h, :],
                            pt[:, glo - tq0 : ghi - tq0],
                            start=(kb == 0),
                            stop=(kb == 4 * qi + 3),
                            skip_group_check=True,
                        )
                        qc += 512

            pending = None  # (h, kb) whose PV is not yet emitted
            for h in range(HC):
                ti, po_ = h // 2, (h % 2) * DK
                po_quarters[h] = [
                    out_psum.tile([DK + 1, 512], FP32, tag="po", name=f"po{h}_{qi}")
                    for qi in range(4)
                ]
                for kb in range(NKB):
                    emit_scores_exp(h, kb)
                    if pending is not None:
                        ph, pkb = pending
                        emit_pv(ph, pkb)
                        pti, ppo_ = ph // 2, (ph % 2) * DK
                        if pkb % 4 == 3:  # quarter pkb//4 just closed
                            normalize(po_quarters[ph][pkb // 4], pkb // 4, pti, ppo_)
                        # last head: trickle one output projection per
                        # iteration as soon as its quarter normalizes
                        # (tb k-4 needs quarter (k-4)//4, closed at
                        # iteration 4*((k-4)//4)+4 <= k).
                        if ph == HC - 1 and pkb >= 4:
                            oproj(pkb - 4, pin=True)
                    pending = (h, kb)
            # flush: last PV closes quarter 3 of the last head; normalize
            # it and emit the remaining output projections.
            emit_pv(HC - 1, NKB - 1)
            oproj(11, pin=True)
            normalize(
                po_quarters[HC - 1][3], 3, (HC - 1) // 2, ((HC - 1) % 2) * DK
            )
            for tb in range(12, NKB):
                oproj(tb, pin=False)

    nc.compile()
    return nc


def _get_nc():
    global _CACHED_NC
    if _CACHED_NC is None:
        _CACHED_NC = build_program()
    return _CACHED_NC


def make_in_maps(Q, K, V, Wq, bq, Wk, bk, Wv, bv, Wo, bo):
    f32 = lambda a: np.ascontiguousarray(a, dtype=np.float32)
    mm = lambda a: np.ascontiguousarray(np.asarray(a), dtype=MM_NP)

    def pack_w(W, cs):
        """[512, 256] slice -> partition-contiguous [128, 4, 256]"""
        w = np.asarray(W)[:, cs].reshape(DM // P, P, COLS).transpose(1, 0, 2)
        return mm(w)

    def pack_o(W, cs):
        """[256, 512] slice -> partition-contiguous [128, 2, 512]"""
        w = np.asarray(W)[cs, :].reshape(COLS // P, P, DM).transpose(1, 0, 2)
        return mm(w)

    def pack_b(b, cs):
        return f32(np.asarray(b)[cs].reshape(COLS // P, P).T)
    qt = [mm(np.asarray(Q[b]).T) for b in range(B)]
    kt = [mm(np.asarray(K[b]).T) for b in range(B)]
    vt = [mm(np.asarray(V[b]).T) for b in range(B)]
    idnmsk = np.empty((P, 2, P), dtype=MM_NP)
    idnmsk[:, 0, :] = np.eye(P, dtype=MM_NP)
    idnmsk[:, 1, :] = np.tril(
        np.full((P, P), -30000.0, dtype=np.float32), -1
    ).astype(MM_NP)
    maps = []
    for c in range(8):
        b, hg = c % B, c // B
        cs = slice(hg * COLS, (hg + 1) * COLS)
        maps.append(
            {
                "QT": qt[b],
                "KT": kt[b],
                "VT": vt[b],
                "WQ": pack_w(Wq, cs),
                "WK": pack_w(Wk, cs),
                "WV": pack_w(Wv, cs),
                "WO": pack_o(Wo, cs),
                "IDNMSK": idnmsk,
                "BQKV": np.concatenate(
                    [
                        pack_b(bq, cs),
                        pack_b(bk, cs),
                        np.broadcast_to(f32(bv[cs]), (P, COLS)),
                    ],
                    axis=1,
                ).astype(np.float32),
            }
        )
    return maps


def assemble(results, bo):
    out = np.empty((B, S, DM), dtype=np.float32)
    for b in range(B):
        out[b] = results[b]["OUT"].astype(np.float32) + results[b + B][
            "OUT"
        ].astype(np.float32)
    out += np.asarray(bo, dtype=np.float32)
    return out


def kernel(Q, K, V, Wq, bq, Wk, bk, Wv, bv, Wo, bo):
    nc = _get_nc()
    maps = make_in_maps(Q, K, V, Wq, bq, Wk, bk, Wv, bv, Wo, bo)
    res = run_bass_kernel_spmd(nc, maps, list(range(8)))
    return assemble(res.results, bo)
